# revision 1
# baseline (speedup 1.0000x reference)
"""DualStreamTemporalModel Trainium2 kernel.

Architecture (per core, SPMD over 8 cores, core c handles batch b = c % 4):
  - 2-layer LSTM (H=256) over T=2048, layers interleaved with 1-chunk lag.
    Gates computed transposed ([gate_dim x 1] tiles, weights stationary bf16).
  - TemporalConv branch (Conv1d 64->256 k=5 + folded BN + SiLU) as tap-matmuls.
  - Attention collapses to the last query row (only context[:, -1] feeds the
    head): k/v projections over all T, one softmax row per head, one AllGather
    of per-sample context vectors, then the MLP head computed redundantly.
"""
import sys
sys.path.insert(0, '/opt/trn_rl_repo')
import numpy as np
import concourse.bass as bass
import concourse.bacc as bacc
import concourse.tile as tile
import concourse.mybir as mybir
from concourse.bass_utils import run_bass_kernel_spmd

F32, BF16 = mybir.dt.float32, mybir.dt.bfloat16
AF = mybir.ActivationFunctionType
OP = mybir.AluOpType
ds = bass.ds

B, T_FULL, IN, H, HEADS, KCONV = 4, 2048, 64, 256, 8, 5
D = 2 * H
EPS = 1e-5
N_CORES = 8
CH = 128  # chunk (steps per loop body)

# torch gate order i,f,g,o -> ours [g, i, f, o]
GPERM = np.r_[2 * H:3 * H, 0:H, H:2 * H, 3 * H:4 * H]


# Blob packing: every weight tensor lives in one [128, BLOB_W] f32 input.
BLOB_SPEC = [
    ("whh0", 128, 2048), ("whh1", 128, 2048), ("wih1", 128, 2048),
    ("wih0", 64, 1024), ("bias0", 128, 8), ("bias1", 128, 8),
    ("ident", 128, 128), ("convw", 64, 1280), ("convb", 128, 2),
    ("wqT", 128, 2048), ("wkT", 128, 2048), ("wpT", 128, 2048),
    ("wvT", 128, 2048), ("qbias", 128, 4), ("kbias", 128, 4),
    ("pbiasT", 128, 4), ("lngT", 128, 4), ("lnbT", 128, 4),
    ("wfc1", 128, 1024), ("fc1b", 128, 2), ("wfc2", 128, 6),
    ("fc2b", 1, 3),
]
BLOB_OFF = {}
_off = 0
for _n, _p, _c in BLOB_SPEC:
    BLOB_OFF[_n] = _off
    _off += _c
BLOB_W = _off


def pack_blob(d):
    blob = np.zeros((128, BLOB_W), np.float32)
    for n, p, c in BLOB_SPEC:
        blob[0:p, BLOB_OFF[n]:BLOB_OFF[n] + c] = d[n]
    return blob


def prep_inputs(inp):
    """numpy weight preprocessing -> (shared input dict, per-core extras)."""
    f32 = lambda a: np.ascontiguousarray(np.asarray(a, np.float32))
    out = {}
    # LSTM weights. whh{l}: [128, 16*128], col block (k*8+m); lhsT tiles of
    # W_hh.T (gate-permuted). wih1 same packing. wih0: [64, 8*128] f32.
    for l in (0, 1):
        whh = f32(inp[f"w_hh{l}"])[GPERM]            # [1024, 256]
        whhT = whh.T                                  # [256, 1024]
        tiles = whhT.reshape(2, 128, 8, 128).transpose(1, 0, 2, 3).reshape(128, 2048)
        out[f"whh{l}"] = tiles
        bsum = f32(inp[f"b_ih{l}"] + inp[f"b_hh{l}"])[GPERM]
        out[f"bias{l}"] = np.ascontiguousarray(bsum.reshape(8, 128).T)  # [128, 8]
    wih0 = f32(inp["w_ih0"])[GPERM]                   # [1024, 64]
    out["wih0"] = np.ascontiguousarray(wih0.T)        # [64, 1024] f32
    wih1 = f32(inp["w_ih1"])[GPERM]                   # [1024, 256]
    out["wih1"] = wih1.T.reshape(2, 128, 8, 128).transpose(1, 0, 2, 3).reshape(128, 2048)
    out["ident"] = np.eye(128, dtype=np.float32)
    # Conv + folded BN.
    s = f32(inp["bn_g"]) / np.sqrt(f32(inp["bn_var"]) + EPS)
    wc = f32(inp["conv_w"]) * s[:, None, None]        # [256, 64, 5]
    bc = (f32(inp["conv_b"]) - f32(inp["bn_mean"])) * s + f32(inp["bn_b"])
    # convw: [64, 5*256]; col = tap*256 + oc
    convw = np.zeros((64, 5 * 256), np.float32)
    for tap in range(5):
        convw[:, tap * 256:(tap + 1) * 256] = wc[:, :, tap].T
    out["convw"] = convw
    out["convb"] = np.ascontiguousarray(bc.reshape(2, 128).T)  # [128, 2]
    # Attention.
    qkv_w = f32(inp["qkv_w"]); qkv_b = f32(inp["qkv_b"])
    Wq, Wk, Wv = qkv_w[0:D], qkv_w[D:2 * D], qkv_w[2 * D:3 * D]
    qb, kb, vb = qkv_b[0:D], qkv_b[D:2 * D], qkv_b[2 * D:3 * D]
    sc = (D // HEADS) ** -0.5
    Wq = Wq * sc; qb = qb * sc

    def packT(W):  # W [512,512] -> lhsT tiles of W.T: [128, (kk*4+m)*128]
        WT = W.T  # [512, 512]
        return np.ascontiguousarray(
            WT.reshape(4, 128, 4, 128).transpose(1, 0, 2, 3).reshape(128, 16 * 128))
    out["wqT"] = packT(Wq)
    out["wkT"] = packT(Wk)
    out["wpT"] = packT(f32(inp["proj_w"]))
    out["wvT"] = np.ascontiguousarray(Wv.T.reshape(4, 128, 512).transpose(1, 0, 2).reshape(128, 4 * 512))
    out["qbias"] = np.ascontiguousarray(qb.reshape(4, 128).T)   # [128,4]
    out["kbias"] = np.ascontiguousarray(kb.reshape(4, 128).T)
    pb_eff = f32(inp["proj_b"]) + vb @ f32(inp["proj_w"]).T
    out["pbiasT"] = np.ascontiguousarray(pb_eff.reshape(4, 128).T)
    # Head.
    out["lngT"] = np.ascontiguousarray(f32(inp["ln_g"]).reshape(4, 128).T)
    out["lnbT"] = np.ascontiguousarray(f32(inp["ln_b"]).reshape(4, 128).T)
    fc1w = f32(inp["fc1_w"])   # [256, 512]
    out["wfc1"] = np.ascontiguousarray(
        fc1w.T.reshape(4, 128, 2, 128).transpose(1, 0, 2, 3).reshape(128, 8 * 128))
    out["fc1b"] = np.ascontiguousarray(f32(inp["fc1_b"]).reshape(2, 128).T)  # [128,2]
    fc2w = f32(inp["fc2_w"])   # [3, 256]
    out["wfc2"] = np.ascontiguousarray(
        fc2w.T.reshape(2, 128, 3).transpose(1, 0, 2).reshape(128, 6))
    out["fc2b"] = f32(inp["fc2_b"])[None, :]   # [1,3]
    return out


def build_nc(T=T_FULL, with_attn=True, dbg_ring=False, attn_stage=99, ch=None):
    global CH
    if ch is not None:
        CH = ch
    NCH = T // CH
    nc = bacc.Bacc("TRN2", target_bir_lowering=False, debug=False,
                   num_devices=N_CORES)
    # ---- DRAM I/O ----
    d_xb = nc.dram_tensor("xb", [max(T, T_FULL), IN], F32, kind="ExternalInput")
    d_blob = nc.dram_tensor("wblob", [128, BLOB_W], F32, kind="ExternalInput")

    class _BlobView:
        def __getitem__(self, name):
            off = BLOB_OFF[name]
            for n, p, c in BLOB_SPEC:
                if n == name:
                    return d_blob[0:p, off:off + c]
            raise KeyError(name)
    d_in = _BlobView()
    d_out = nc.dram_tensor("out", [1, 3], F32, kind="ExternalOutput")
    if dbg_ring:
        d_dbg = nc.dram_tensor("dbg_ring", [128, 2 * T], F32, kind="ExternalOutput")

    with tile.TileContext(nc) as tc:
        import contextlib
        stack = contextlib.ExitStack()
        with stack:
            sb = stack.enter_context(tc.tile_pool(name="sb", bufs=1))
            dma2 = stack.enter_context(tc.tile_pool(name="dma2", bufs=2))
            lstm_ps = contextlib.ExitStack()
            psg = lstm_ps.enter_context(tc.tile_pool(name="psg", bufs=2, space="PSUM"))
            psA = lstm_ps.enter_context(tc.tile_pool(name="psA", bufs=1, space="PSUM"))
            psB = lstm_ps.enter_context(tc.tile_pool(name="psB", bufs=1, space="PSUM"))

            # ---- persistent SBUF ----
            t_whh = [sb.tile([128, 2048], BF16, name=f"whh{l}_t", tag=f"whh{l}") for l in (0, 1)]
            t_wih1 = sb.tile([128, 2048], BF16, name="t001")
            t_wih0 = sb.tile([64, 1024], F32, name="t002")
            t_bias = [sb.tile([128, 8], F32, name=f"bias{l}_t", tag=f"bias{l}") for l in (0, 1)]
            t_id = sb.tile([128, 128], F32, name="t003")
            ring1 = sb.tile([128, 2 * T], BF16, name="t004")           # lstm_out.T packed (t,k)
            hb = [sb.tile([128, 2 * CH + 2], BF16, name=f"hbuf{l}", tag=f"hb{l}") for l in (0, 1)]
            hb0p = sb.tile([128, 2 * CH + 2], BF16, name="t005")       # prev chunk of layer0
            gxb = [sb.tile([128, 8 * CH], F32, name=f"gxbuf{l}", tag=f"gx{l}") for l in (0, 1)]
            S = [sb.tile([128, 4], F32, name=f"state{l}", tag=f"S{l}") for l in (0, 1)]   # [g0,g1,c0,c1]
            sgb = [sb.tile([128, 6], F32, name=f"sgbuf{l}", tag=f"sg{l}") for l in (0, 1)]
            Pb = [sb.tile([128, 4], F32, name=f"pbuf{l}", tag=f"P{l}") for l in (0, 1)]
            thb = [sb.tile([128, 2], F32, name=f"thbuf{l}", tag=f"th{l}") for l in (0, 1)]

            # weight DMAs (bf16 via staging copy)
            def load_bf16(dst, src_dram):
                stg = dma2.tile(list(src_dram.shape), F32, tag="stg")
                nc.sync.dma_start(stg[:], src_dram[:])
                nc.vector.tensor_copy(dst[:], stg[:])
            load_bf16(t_whh[0], d_in["whh0"])
            load_bf16(t_whh[1], d_in["whh1"])
            load_bf16(t_wih1, d_in["wih1"])
            nc.sync.dma_start(t_wih0[:], d_in["wih0"][:])
            nc.sync.dma_start(t_bias[0][:], d_in["bias0"][:])
            nc.sync.dma_start(t_bias[1][:], d_in["bias1"][:])
            nc.sync.dma_start(t_id[:], d_in["ident"][:])
            nc.gpsimd.memset(hb[0][:, 0:2], 0.0)
            nc.gpsimd.memset(hb[1][:, 0:2], 0.0)
            nc.gpsimd.memset(S[0][:, 2:4], 0.0)
            nc.gpsimd.memset(S[1][:, 2:4], 0.0)

            xbT = d_xb.rearrange("t c -> c t")  # dram view [64, T]

            def emit_gx0(t0_expr):
                """gate pre-activations from x for chunk starting at t0."""
                xt = dma2.tile([64, CH], F32, tag="xt", name="t006")
                nc.sync.dma_start(xt[:], xbT[:, ds(t0_expr, CH)])
                for m in range(8):
                    pg = psg.tile([128, CH], F32, tag="pg", name="t007")
                    nc.tensor.matmul(pg[:], t_wih0[:, m * 128:(m + 1) * 128],
                                     xt[:], start=True, stop=True)
                    nc.vector.tensor_scalar_add(gxb[0][:, ds(m, CH, 8)], pg[:],
                                                t_bias[0][:, m:m + 1])

            def emit_gx1():
                """layer-1 input projections from hb0p (prev chunk of layer0)."""
                for m in range(8):
                    pg = psg.tile([128, CH], F32, tag="pg", name="t008")
                    for k in range(2):
                        nc.tensor.matmul(
                            pg[:], t_wih1[:, (k * 8 + m) * 128:(k * 8 + m + 1) * 128],
                            hb0p[:, ds(2 + k, CH, 2)],
                            start=(k == 0), stop=(k == 1))
                    nc.vector.tensor_scalar_add(gxb[1][:, ds(m, CH, 8)], pg[:],
                                                t_bias[1][:, m:m + 1])

            def step_mm(l, tl):
                pA = psA.tile([128, 2], F32, tag=f"pA{l}", name="t009")
                pB = psB.tile([128, 6], F32, tag=f"pB{l}", name="t010")
                gx = gxb[l]
                nc.tensor.matmul(pA[:], t_id[:], gx[:, 8 * tl:8 * tl + 2],
                                 start=True, stop=False)
                nc.tensor.matmul(pB[:], t_id[:], gx[:, 8 * tl + 2:8 * tl + 8],
                                 start=True, stop=False)
                w = t_whh[l]
                hsrc = hb[l]
                for m in range(8):
                    ps, col = (pA, m) if m < 2 else (pB, m - 2)
                    for k in range(2):
                        nc.tensor.matmul(
                            ps[:, col:col + 1],
                            w[:, (k * 8 + m) * 128:(k * 8 + m + 1) * 128],
                            hsrc[:, 2 * tl + k:2 * tl + k + 1],
                            start=False,
                            stop=(k == 1 and (m == 1 or m == 7)))
                return pA, pB

            def step_tail(pp, tl, phase):
                if phase == 0:
                    for l, (pA, pB) in pp:
                        nc.scalar.activation(S[l][:, 0:2], pA[:], AF.Tanh)
                        nc.scalar.activation(sgb[l][:], pB[:], AF.Sigmoid)
                elif phase == 1:
                    for l, _ in pp:
                        nc.vector.tensor_mul(Pb[l][:], sgb[l][:, 0:4], S[l][:, 0:4])
                        nc.vector.tensor_add(S[l][:, 2:4], Pb[l][:, 0:2], Pb[l][:, 2:4])
                elif phase == 2:
                    for l, _ in pp:
                        nc.scalar.activation(thb[l][:], S[l][:, 2:4], AF.Tanh)
                else:
                    for l, _ in pp:
                        nc.vector.tensor_mul(hb[l][:, 2 * tl + 2:2 * tl + 4],
                                             sgb[l][:, 4:6], thb[l][:])

            def emit_step(l, tl):
                pp = [(l, step_mm(l, tl))]
                for ph in range(4):
                    step_tail(pp, tl, ph)

            def emit_step2(tl):
                pp = [(0, step_mm(0, tl)), (1, step_mm(1, tl))]
                for ph in range(4):
                    step_tail(pp, tl, ph)

            def carry(l):
                nc.vector.tensor_copy(hb[l][:, 0:2], hb[l][:, 2 * CH:2 * CH + 2])

            # ---- peel: chunk 0 of layer 0 ----
            emit_gx0(0)
            for tl in range(CH):
                emit_step(0, tl)
            nc.vector.tensor_copy(hb0p[:], hb[0][:])
            carry(0)

            # ---- main loop: j = 1..NCH-1 ----
            if NCH > 1:
                with tc.For_i(1, NCH) as iv:
                    emit_gx0(iv * CH)
                    emit_gx1()
                    for tl in range(CH):
                        emit_step2(tl)
                    nc.vector.tensor_copy(ring1[:, ds(iv * (2 * CH) - 2 * CH, 2 * CH)],
                                          hb[1][:, 2:2 * CH + 2])
                    nc.vector.tensor_copy(hb0p[:], hb[0][:])
                    carry(0)
                    carry(1)

            # ---- epilogue: last chunk of layer 1 ----
            emit_gx1()
            for tl in range(CH):
                emit_step(1, tl)
            nc.vector.tensor_copy(ring1[:, (NCH - 1) * 2 * CH:NCH * 2 * CH],
                                  hb[1][:, 2:2 * CH + 2])

            lstm_ps.close()

            if dbg_ring:
                rf = sb.tile([128, 2 * T], F32, name="t011")
                nc.vector.tensor_copy(rf[:], ring1[:])
                nc.sync.dma_start(d_dbg[:], rf[:])

            if with_attn:
                emit_attn(nc, tc, stack, sb, dma2, d_in, d_xb, d_out,
                          ring1, t_id, T, attn_stage)
    nc.compile()
    return nc


def emit_attn(nc, tc, stack, sb, dma2, d_in, d_xb, d_out,
              ring1, t_id, T, attn_stage=99):
    NT512 = T // 512
    NT128 = T // 128
    ps512 = stack.enter_context(tc.tile_pool(name="ps512", bufs=2, space="PSUM"))
    pssm = stack.enter_context(tc.tile_pool(name="pssm", bufs=2, space="PSUM"))
    psc = stack.enter_context(tc.tile_pool(name="psc", bufs=2, space="PSUM"))
    psv = stack.enter_context(tc.tile_pool(name="psv", bufs=1, space="PSUM"))

    # weights
    t_convw = sb.tile([64, 1280], F32, name="t012")
    nc.sync.dma_start(t_convw[:], d_in["convw"][:])
    t_convb = sb.tile([128, 2], F32, name="t013")
    nc.sync.dma_start(t_convb[:], d_in["convb"][:])
    wT = {}
    for nm in ("wqT", "wkT", "wpT", "wvT"):
        wT[nm] = sb.tile([128, 2048], BF16, name=f"wt_{nm}", tag=nm)
        stg = dma2.tile([128, 2048], F32, tag="stg2", name="t014")
        nc.sync.dma_start(stg[:], d_in[nm][:])
        nc.vector.tensor_copy(wT[nm][:], stg[:])
    t_qb = sb.tile([128, 4], F32, name="t015"); nc.sync.dma_start(t_qb[:], d_in["qbias"][:])
    t_kb = sb.tile([128, 4], F32, name="t016"); nc.sync.dma_start(t_kb[:], d_in["kbias"][:])
    t_pbT = sb.tile([128, 4], F32, name="t017"); nc.sync.dma_start(t_pbT[:], d_in["pbiasT"][:])

    # ---- conv branch: convT [128, 2*T] bf16 (col = oc*T + t) ----
    convT = sb.tile([128, 2 * T], BF16, name="t018")
    xpad = sb.tile([64, T + 4], F32, name="t019")
    nc.gpsimd.memset(xpad[:, 0:2], 0.0)
    nc.gpsimd.memset(xpad[:, T + 2:T + 4], 0.0)
    nc.sync.dma_start(xpad[:, 2:T + 2], d_xb[0:T, :].rearrange("t c -> c t"))
    for oc in range(2):
        for tb in range(NT512):
            pc = ps512.tile([128, 512], F32, tag="p512", name="t020")
            for tap in range(5):
                nc.tensor.matmul(
                    pc[:], t_convw[:, tap * 256 + oc * 128:tap * 256 + oc * 128 + 128],
                    xpad[:, tb * 512 + tap:tb * 512 + tap + 512],
                    start=(tap == 0), stop=(tap == 4))
            sg = dma2.tile([128, 512], F32, tag="csg", name="t021")
            nc.scalar.activation(sg[:], pc[:], AF.Sigmoid, bias=t_convb[:, oc:oc + 1])
            nc.vector.scalar_tensor_tensor(
                convT[:, oc * T + tb * 512:oc * T + tb * 512 + 512],
                pc[:], t_convb[:, oc:oc + 1], sg[:], op0=OP.add, op1=OP.mult)

    def mergedT_tile(kk, c0, n):
        """AP of merged.T tile [128, n] for feature-tile kk, cols t=c0..c0+n."""
        if kk < 2:
            return ring1[:, ds(2 * c0 + kk, n, 2)]
        return convT[:, (kk - 2) * T + c0:(kk - 2) * T + c0 + n]

    if attn_stage < 2:
        return
    # ---- kT projection: kT [128, 4*T] bf16 (col = m*T + t) ----
    kT = sb.tile([128, 4 * T], BF16, name="t022")
    for m in range(4):
        for tb in range(NT512):
            pk = ps512.tile([128, 512], F32, tag="p512", name="t023")
            for kk in range(4):
                nc.tensor.matmul(pk[:],
                                 wT["wkT"][:, (kk * 4 + m) * 128:(kk * 4 + m + 1) * 128],
                                 mergedT_tile(kk, tb * 512, 512),
                                 start=(kk == 0), stop=(kk == 3))
            nc.vector.tensor_scalar_add(kT[:, m * T + tb * 512:m * T + tb * 512 + 512],
                                        pk[:], t_kb[:, m:m + 1])

    if attn_stage < 3:
        return
    # ---- v projection (normal layout): v [128, NT128*512] bf16 ----
    vN = sb.tile([128, NT128 * 512], BF16, name="t024")
    for tb in range(NT128):
        pv = ps512.tile([128, 512], F32, tag="p512", name="t025")
        for kk in range(4):
            nc.tensor.matmul(pv[:], mergedT_tile(kk, tb * 128, 128),
                             wT["wvT"][:, kk * 512:(kk + 1) * 512],
                             start=(kk == 0), stop=(kk == 3))
        nc.vector.tensor_copy(vN[:, tb * 512:(tb + 1) * 512], pv[:])

    if attn_stage < 4:
        return
    # ---- q (last timestep) + blockdiag lhsT ----
    qT = sb.tile([128, 4], F32, name="t026")
    for m in range(4):
        pq = pssm.tile([128, 8], F32, tag="psmall", name="t027")
        for kk in range(4):
            nc.tensor.matmul(pq[:, 0:1],
                             wT["wqT"][:, (kk * 4 + m) * 128:(kk * 4 + m + 1) * 128],
                             mergedT_tile(kk, T - 1, 1),
                             start=(kk == 0), stop=(kk == 3))
        nc.vector.tensor_scalar_add(qT[:, m:m + 1], pq[:, 0:1], t_qb[:, m:m + 1])
    qbd = sb.tile([128, 32], BF16, name="t028")   # col = m*8 + h
    nc.gpsimd.memset(qbd[:], 0.0)
    for h in range(HEADS):
        m, half = h // 2, h % 2
        nc.vector.tensor_copy(qbd[half * 64:half * 64 + 64, m * 8 + h:m * 8 + h + 1],
                              qT[half * 64:half * 64 + 64, m:m + 1])

    if attn_stage < 5:
        return
    # ---- scores [8, T] + softmax ----
    srow = sb.tile([8, T], F32, name="t029")
    for tb in range(NT512):
        sc = psc.tile([8, 512], F32, tag="sc", name="t030")
        for m in range(4):
            nc.tensor.matmul(sc[:], qbd[:, m * 8:(m + 1) * 8],
                             kT[:, m * T + tb * 512:m * T + tb * 512 + 512],
                             start=(m == 0), stop=(m == 3))
        nc.vector.tensor_copy(srow[:, tb * 512:(tb + 1) * 512], sc[:])
    mxr = sb.tile([8, 1], F32, name="t031")
    nc.vector.reduce_max(mxr[:], srow[:], axis=mybir.AxisListType.X)
    negm = sb.tile([8, 1], F32, name="t032")
    nc.vector.tensor_scalar_mul(negm[:], mxr[:], -1.0)
    wrow = sb.tile([8, T], F32, name="t033")
    part = sb.tile([8, NT512], F32, name="t034")
    for tb in range(NT512):
        nc.scalar.activation(wrow[:, tb * 512:(tb + 1) * 512],
                             srow[:, tb * 512:(tb + 1) * 512],
                             AF.Exp, bias=negm[:], accum_out=part[:, tb:tb + 1])
    den = sb.tile([8, 1], F32, name="t035")
    nc.vector.reduce_sum(den[:], part[:], axis=mybir.AxisListType.X)
    rden = sb.tile([8, 1], F32, name="t036")
    nc.vector.reciprocal(rden[:], den[:])
    nc.vector.tensor_scalar_mul(wrow[:], wrow[:], rden[:])
    # transpose weights: wT128 [128, NT128*8] bf16 (col = tb*8 + h)
    wT128 = sb.tile([128, NT128 * 8], BF16, name="t037")
    for tb in range(NT128):
        pt = pssm.tile([128, 8], F32, tag="psmall", name="t038")
        nc.tensor.transpose(pt[:], wrow[:, tb * 128:(tb + 1) * 128], t_id[0:8, 0:8])
        nc.vector.tensor_copy(wT128[:, tb * 8:(tb + 1) * 8], pt[:])

    if attn_stage < 6:
        return
    # ---- attn = sum_t w_t v_t : [8, 512] ----
    pav = psv.tile([8, 512], F32, tag="pav", name="t039")
    for tb in range(NT128):
        nc.tensor.matmul(pav[:], wT128[:, tb * 8:(tb + 1) * 8],
                         vN[:, tb * 512:(tb + 1) * 512],
                         start=(tb == 0), stop=(tb == NT128 - 1))
    av = sb.tile([8, 512], F32, name="t040")
    nc.vector.tensor_copy(av[:], pav[:])
    # diag-extract to attnT [128, 4] bf16 via 4 dma transposes + col selects
    attnT = sb.tile([128, 4], BF16, name="t041")
    for kk in range(4):
        ptr = pssm.tile([128, 8], F32, tag="psmall", name="t042")
        nc.tensor.transpose(ptr[:], av[:, kk * 128:(kk + 1) * 128], t_id[0:8, 0:8])
        nc.vector.tensor_copy(attnT[0:64, kk:kk + 1], ptr[0:64, 2 * kk:2 * kk + 1])
        nc.vector.tensor_copy(attnT[64:128, kk:kk + 1],
                              ptr[64:128, 2 * kk + 1:2 * kk + 2])

    if attn_stage < 7:
        return
    # ---- context vector: proj + pbias(+vb folded) + merged_last ----
    pctx = pssm.tile([128, 8], F32, tag="psmall", name="t043")
    for m in range(4):
        for kk in range(4):
            nc.tensor.matmul(pctx[:, m:m + 1],
                             wT["wpT"][:, (kk * 4 + m) * 128:(kk * 4 + m + 1) * 128],
                             attnT[:, kk:kk + 1],
                             start=(kk == 0), stop=(kk == 3))
    ctxT = sb.tile([128, 4], F32, name="t044")
    nc.vector.tensor_add(ctxT[:], pctx[:, 0:4], t_pbT[:])
    for m in range(4):
        nc.vector.tensor_add(ctxT[:, m:m + 1], ctxT[:, m:m + 1],
                             mergedT_tile(m, T - 1, 1))
    # -> DRAM, AllGather
    if attn_stage < 8:
        return
    # ---- per-sample LayerNorm + head, all in transposed layout ----
    t_lngT = sb.tile([128, 4], F32, name="lngT")
    nc.sync.dma_start(t_lngT[:], d_in["lngT"][:])
    t_lnbT = sb.tile([128, 4], F32, name="lnbT")
    nc.sync.dma_start(t_lnbT[:], d_in["lnbT"][:])
    t_wfc1 = sb.tile([128, 1024], BF16, name="wfc1t")
    stg3 = dma2.tile([128, 1024], F32, name="stg3", tag="stg3")
    nc.sync.dma_start(stg3[:], d_in["wfc1"][:])
    nc.vector.tensor_copy(t_wfc1[:], stg3[:])
    t_fc1b = sb.tile([128, 2], F32, name="fc1bt")
    nc.sync.dma_start(t_fc1b[:], d_in["fc1b"][:])
    t_wfc2 = sb.tile([128, 6], F32, name="wfc2t")
    nc.sync.dma_start(t_wfc2[:], d_in["wfc2"][:])
    t_fc2b = sb.tile([1, 3], F32, name="fc2bt")
    nc.sync.dma_start(t_fc2b[:], d_in["fc2b"][:])
    ones_col = sb.tile([128, 1], F32, name="ones_col")
    nc.gpsimd.memset(ones_col[:], 1.0)
    ones_row = sb.tile([1, 128], F32, name="ones_row")
    nc.gpsimd.memset(ones_row[:], 1.0)

    # mean / var via ones-matmuls (cross-partition sums)
    csq = sb.tile([128, 4], F32, name="csq")
    nc.vector.tensor_mul(csq[:], ctxT[:], ctxT[:])
    psums = pssm.tile([128, 8], F32, tag="psmall", name="pl1")
    nc.tensor.matmul(psums[0:1, 0:4], ones_col[:], ctxT[:], start=True, stop=False)
    nc.tensor.matmul(psums[0:1, 4:8], ones_col[:], csq[:], start=False, stop=True)
    srow = sb.tile([1, 8], F32, name="lnsrow")
    nc.vector.tensor_copy(srow[:], psums[0:1, 0:8])
    mu1 = sb.tile([1, 1], F32, name="mu1")
    nc.vector.reduce_sum(mu1[:], srow[:, 0:4], axis=mybir.AxisListType.X)
    nc.vector.tensor_scalar_mul(mu1[:], mu1[:], 1.0 / 512)
    sq1 = sb.tile([1, 1], F32, name="sq1")
    nc.vector.reduce_sum(sq1[:], srow[:, 4:8], axis=mybir.AxisListType.X)
    # var = E[x^2] - mu^2 ; rstd = 1/sqrt(var+eps)
    var1 = sb.tile([1, 1], F32, name="var1")
    nc.vector.scalar_tensor_tensor(var1[:], mu1[:], -1.0, mu1[:],
                                   op0=OP.mult, op1=OP.mult)
    nc.vector.scalar_tensor_tensor(var1[:], sq1[:], 1.0 / 512, var1[:],
                                   op0=OP.mult, op1=OP.add)
    epst = sb.tile([1, 1], F32, name="epst")
    nc.gpsimd.memset(epst[:], EPS)
    sd1 = sb.tile([1, 1], F32, name="sd1")
    nc.scalar.activation(sd1[:], var1[:], AF.Sqrt, bias=epst[:])
    rsd1 = sb.tile([1, 1], F32, name="rsd1")
    nc.vector.reciprocal(rsd1[:], sd1[:])
    # broadcast mu, rstd to [128,1] via K=1 matmuls
    pbc = pssm.tile([128, 8], F32, tag="psmall", name="pl2")
    nc.tensor.matmul(pbc[:, 0:1], ones_row[:], mu1[:], start=True, stop=False)
    nc.tensor.matmul(pbc[:, 1:2], ones_row[:], rsd1[:], start=False, stop=True)
    mubc = sb.tile([128, 2], F32, name="mubc")
    nc.vector.tensor_copy(mubc[:], pbc[:, 0:2])
    # z = (ctx - mu) * rstd * lng + lnb   (feat on partitions)
    zt = sb.tile([128, 4], F32, name="zt")
    nc.vector.tensor_scalar_sub(zt[:], ctxT[:], mubc[:, 0:1])
    nc.vector.tensor_scalar_mul(zt[:], zt[:], mubc[:, 1:2])
    nc.vector.tensor_mul(zt[:], zt[:], t_lngT[:])
    nc.vector.tensor_add(zt[:], zt[:], t_lnbT[:])
    zb = sb.tile([128, 4], BF16, name="zb")
    nc.vector.tensor_copy(zb[:], zt[:])
    # fc1 + silu (out feat on partitions: 2 m-tiles)
    p1 = pssm.tile([128, 8], F32, tag="psmall", name="pl3")
    for m in range(2):
        for kk in range(4):
            nc.tensor.matmul(p1[:, m:m + 1],
                             t_wfc1[:, (kk * 2 + m) * 128:(kk * 2 + m + 1) * 128],
                             zb[:, kk:kk + 1], start=(kk == 0), stop=(kk == 3))
    h1T = sb.tile([128, 2], F32, name="h1T")
    sg1 = sb.tile([128, 2], F32, name="sg1h")
    for m in range(2):
        nc.scalar.activation(sg1[:, m:m + 1], p1[:, m:m + 1], AF.Sigmoid,
                             bias=t_fc1b[:, m:m + 1])
        nc.vector.scalar_tensor_tensor(h1T[:, m:m + 1], p1[:, m:m + 1],
                                       t_fc1b[:, m:m + 1], sg1[:, m:m + 1],
                                       op0=OP.add, op1=OP.mult)
    # fc2: out [1, 3]
    p2 = pssm.tile([128, 8], F32, tag="psmall", name="pl4")
    for kk in range(2):
        nc.tensor.matmul(p2[0:1, 0:3], h1T[:, kk:kk + 1],
                         t_wfc2[:, kk * 3:(kk + 1) * 3],
                         start=(kk == 0), stop=(kk == 1))
    lg = sb.tile([1, 3], F32, name="lgt")
    nc.vector.tensor_add(lg[:], p2[0:1, 0:3], t_fc2b[:])
    ob = sb.tile([1, 3], F32, name="obt")
    nc.scalar.activation(ob[:, 0:1], lg[:, 0:1], AF.Tanh)
    nc.scalar.activation(ob[:, 2:3], lg[:, 2:3], AF.Sigmoid)
    eu = sb.tile([1, 1], F32, name="eut")
    nc.scalar.activation(eu[:], lg[:, 1:2], AF.Exp)
    nc.scalar.activation(ob[:, 1:2], eu[:], AF.Ln, bias=1.0)
    nc.sync.dma_start(d_out[:], ob[:])


_NC_CACHE = {}


def kernel(**inputs):
    key = "full"
    if key not in _NC_CACHE:
        _NC_CACHE[key] = build_nc(T=T_FULL, with_attn=True)
    nc = _NC_CACHE[key]
    blob = pack_blob(prep_inputs(inputs))
    x = np.asarray(inputs["x"], np.float32)
    in_maps = [{"wblob": blob, "xb": np.ascontiguousarray(x[c % 4])}
               for c in range(N_CORES)]
    res = run_bass_kernel_spmd(nc, in_maps, list(range(N_CORES)))
    outs = np.stack([res.results[b]["out"][0] for b in range(4)])  # [4,3]
    return outs[:, 0], outs[:, 1], outs[:, 2]


if __name__ == "__main__":
    pass

